# revision 5
# baseline (speedup 1.0000x reference)
"""Trainium2 Bass kernel for nn_AttnDecoder (B=8192, T=10, CH=H=512).

Math notes (verified against the jax reference in fp32 to ~3e-6):
  - The attention block is dead code: softmax over a size-1 axis == 1, so
    h1 == ht and attn1/2/3 never affect the output.
  - The LSTM hidden state d never feeds back into the gates (only the cell
    state c does, elementwise), so the only sequential part is
        c_t = sigmoid(f_t) * c_{t-1} + sigmoid(i_t) * tanh(g_t)
    which maps 1:1 onto the DVE tensor_tensor_scan instruction (fp32 state).
  - o-gate is only needed at t = T-1.
  - fc2(fc1(z)) with no nonlinearity folds into a single vector:
        y = d . v[:H] + h9 . v[H:] + c0,   v = (fc2_w @ fc1_w)^T.

Sharding: batch-parallel over 8 cores (1024 rows each), weights replicated.

Device pipeline per core:
  1. SWDGE cast-DMA: h fp32 (DRAM) -> bf16 DRAM staging, per (batch-group, t).
  2. HWDGE xbar transpose-DMA: staging -> SBUF hT [ch, batch] bf16 tiles.
  3. PE: gates^T = W_ifg @ hT (bf16, fp32 accum) + K=1 rank-1 matmul for the
     y_t (x) w_ih term, into PSUM [gate, batch].
  4. ACT: sigmoid/tanh straight from PSUM (both funcs share one table set),
     with the gate bias applied as the per-partition activation bias.
  5. DVE: m = sigma(i)*tanh(g); tensor_tensor_scan(mult, add) runs the whole
     c recurrence along the free dim (layout [hid, (batch, t)], t innermost;
     sigma(f) is zeroed at t=0 columns so the carry resets per batch row).
  6. Final: d = sigma(o)*tanh(c9); y_d via fp32 PE dot with v[:H];
     y_h = h9 . v[H:] in fp32 on DVE from naturally-laid-out h9;
     the two partial outputs are summed on the host during unsharding.
"""

import numpy as np
import ml_dtypes

import concourse.bass as bass
import concourse.tile as tile
from concourse import bacc, mybir
from concourse.bass_utils import run_bass_kernel_spmd

BF16 = ml_dtypes.bfloat16

B, T, CH, H = 8192, 10, 512, 512
N_CORES = 8
B_LOC = B // N_CORES            # 1024 batch rows per core
P = 128

_compiled = {}


def build_nc(b_loc=B_LOC, bgrp=512, psum_bufs=6, hT_bufs=2):
    NBG = b_loc // bgrp         # batch groups
    NJ = H // P                 # 4 hid chunks
    NK = CH // P                # 4 contraction chunks
    NBT = b_loc // P            # batch tiles for the h9 dot
    f32 = mybir.dt.float32
    bf16 = mybir.dt.bfloat16
    AF = mybir.ActivationFunctionType
    ALU = mybir.AluOpType

    nc = bacc.Bacc("TRN2", target_bir_lowering=False, debug=False,
                   num_devices=N_CORES)

    h_in = nc.dram_tensor("h", [b_loc, T, CH], f32, kind="ExternalInput")
    yT_in = nc.dram_tensor("yT", [T, b_loc], bf16, kind="ExternalInput")
    # W_all[k][r, col] = W_hh[col, 128k + r] (bf16), col spans i,f,g,o = 2048
    w_in = nc.dram_tensor("w_all", [NK, P, 4 * H], bf16, kind="ExternalInput")
    wih_in = nc.dram_tensor("wih", [1, 4 * H], bf16, kind="ExternalInput")
    bias_in = nc.dram_tensor("bias", [P, 4 * H // P], f32, kind="ExternalInput")
    vd_in = nc.dram_tensor("v_d", [P, NJ], f32, kind="ExternalInput")
    vh_in = nc.dram_tensor("v_h", [P, CH], f32, kind="ExternalInput")
    outd = nc.dram_tensor("out_d", [b_loc], f32, kind="ExternalOutput")
    outh = nc.dram_tensor("out_h", [P, NBT], f32, kind="ExternalOutput")

    h_ap = h_in.ap()

    # gate order in the 2048-wide W columns: i(0:512) f(512:1024) g(1024:1536) o(1536:2048)
    M_I, M_F, M_G, M_O = 0, NJ, 2 * NJ, 3 * NJ

    with tile.TileContext(nc) as tc:
        with (
            tc.tile_pool(name="const", bufs=1) as const,
            tc.tile_pool(name="dram", bufs=1, space="DRAM") as dram,
            tc.tile_pool(name="hT", bufs=hT_bufs) as hTp,
            tc.tile_pool(name="work", bufs=3) as work,
            tc.tile_pool(name="scan", bufs=1) as scanp,
            tc.tile_pool(name="fin", bufs=2) as fin,
            tc.tile_pool(name="psum", bufs=psum_bufs, space="PSUM") as psum,
            tc.tile_pool(name="psum_y", bufs=1, space="PSUM") as psum_y,
        ):
            # ---- weights / constants into SBUF ----
            w_sb = []
            for k in range(NK):
                wt = const.tile([P, 4 * H], bf16, name=f"w_sb{k}", tag=f"w{k}")
                nc.sync.dma_start(wt[:], w_in.ap()[k])
                w_sb.append(wt)
            wih_sb = const.tile([1, 4 * H], bf16, name="wih_sb")
            nc.sync.dma_start(wih_sb[:], wih_in.ap())
            bias_sb = const.tile([P, 4 * H // P], f32, name="bias_sb")
            nc.sync.dma_start(bias_sb[:], bias_in.ap())
            vd_sb = const.tile([P, NJ], f32, name="vd_sb")
            nc.sync.dma_start(vd_sb[:], vd_in.ap())
            vh_sb = const.tile([P, CH], f32, name="vh_sb")
            nc.sync.dma_start(vh_sb[:], vh_in.ap())
            yt_sb = const.tile([1, T * b_loc], bf16, name="yt_sb")
            nc.sync.dma_start(yt_sb[:], yT_in.ap().rearrange("t b -> (t b)")[None, :])

            # ---- stage 1: cast h to bf16 in DRAM (SWDGE cast DMA) ----
            staged = [[None] * T for _ in range(NBG)]
            for bg in range(NBG):
                bs = bg * bgrp
                for t in range(T):
                    st = dram.tile([bgrp, CH], bf16, name=f"stg_{bg}_{t}",
                                   tag=f"stg_{bg}_{t}")
                    nc.gpsimd.dma_start(st[:], h_ap[bs:bs + bgrp, t, :])
                    staged[bg][t] = st

            # ---- main loop ----
            for bg in range(NBG):
                sf_t, m_t = [], []
                for j in range(NJ):
                    sf = scanp.tile([P, bgrp * T], bf16, name=f"sf{j}",
                                    tag=f"sf{j}")
                    m = scanp.tile([P, bgrp * T], bf16, name=f"m{j}",
                                   tag=f"m{j}")
                    # zero sigmoid(f) at t=0 columns: carry reset per batch row
                    nc.vector.memset(
                        sf.rearrange("p (b t) -> p b t", t=T)[:, :, 0], 0.0)
                    sf_t.append(sf)
                    m_t.append(m)
                so_t = [None] * NJ
                hT9 = [None] * NK

                for t in range(T):
                    # transpose-DMA h^T chunks for this (bg, t)
                    hT = []
                    for k in range(NK):
                        ht = hTp.tile([P, bgrp], bf16, name=f"hT{k}",
                                      tag=f"hT{k}")
                        nc.sync.dma_start_transpose(
                            ht[:], staged[bg][t][:, k * P:(k + 1) * P])
                        hT.append(ht)
                    if t == T - 1:
                        hT9 = hT

                    if t == 0:
                        gates = ("i", "g")
                    elif t == T - 1:
                        gates = ("i", "g", "f", "o")
                    else:
                        gates = ("i", "g", "f")

                    for j in range(NJ):
                        acts = {}
                        for gate in gates:
                            m_idx = {"i": M_I, "f": M_F, "g": M_G, "o": M_O}[gate] + j
                            ps = psum.tile([P, bgrp], f32, name="ps_g", tag="ps_g")
                            # K=1 rank-1 matmul: y_t (x) w_ih column block
                            nc.tensor.matmul(
                                ps[:],
                                wih_sb[0:1, m_idx * P:(m_idx + 1) * P],
                                yt_sb[0:1, t * b_loc + bg * bgrp:
                                      t * b_loc + (bg + 1) * bgrp],
                                start=True, stop=False)
                            for k in range(NK):
                                nc.tensor.matmul(
                                    ps[:],
                                    w_sb[k][:, m_idx * P:(m_idx + 1) * P],
                                    hT[k][:],
                                    start=False, stop=(k == NK - 1))
                            acts[gate] = (ps, m_idx)

                        # ACT: psum -> sigma/tanh (+bias) -> SBUF
                        ps, mi = acts["i"]
                        si = work.tile([P, bgrp], f32, name="si", tag="si")
                        nc.scalar.activation(si[:], ps[:], AF.Sigmoid,
                                             bias=bias_sb[:, mi:mi + 1])
                        ps, mi = acts["g"]
                        tg = work.tile([P, bgrp], f32, name="tg", tag="tg")
                        nc.scalar.activation(tg[:], ps[:], AF.Tanh,
                                             bias=bias_sb[:, mi:mi + 1])
                        # m[:, :, t] = si * tg  (bf16 out, strided)
                        m3d = m_t[j].rearrange("p (b t) -> p b t", t=T)
                        nc.vector.tensor_tensor(
                            m3d[:, :, t], si[:], tg[:], ALU.mult)
                        if "f" in gates:
                            ps, mi = acts["f"]
                            sf3d = sf_t[j].rearrange("p (b t) -> p b t", t=T)
                            nc.scalar.activation(sf3d[:, :, t], ps[:],
                                                 AF.Sigmoid,
                                                 bias=bias_sb[:, mi:mi + 1])
                        if "o" in gates:
                            ps, mi = acts["o"]
                            so = fin.tile([P, bgrp], f32, name="so",
                                          tag=f"so{j}", bufs=1)
                            nc.scalar.activation(so[:], ps[:], AF.Sigmoid,
                                                 bias=bias_sb[:, mi:mi + 1])
                            so_t[j] = so

                # ---- scan + finalize this batch group ----
                ps_y = psum_y.tile([1, bgrp], f32, name="ps_y", tag="ps_y")
                for j in range(NJ):
                    # c recurrence: state = sf*state + m, fp32 state, in-place
                    nc.vector.tensor_tensor_scan(
                        m_t[j][:], sf_t[j][:], m_t[j][:], 0.0,
                        ALU.mult, ALU.add)
                    c9 = m_t[j].rearrange("p (b t) -> p b t", t=T)[:, :, T - 1]
                    tc9 = fin.tile([P, bgrp], f32, name="tc9", tag="tc9")
                    nc.scalar.activation(tc9[:], c9, AF.Tanh)
                    d = fin.tile([P, bgrp], f32, name="d", tag="d")
                    nc.vector.tensor_tensor(d[:], so_t[j][:], tc9[:], ALU.mult)
                    nc.tensor.matmul(ps_y[:], vd_sb[:, j:j + 1], d[:],
                                     start=(j == 0), stop=(j == NJ - 1))
                y_d = fin.tile([1, bgrp], f32, name="y_d", tag="y_d")
                nc.scalar.activation(y_d[:], ps_y[:], AF.Copy, bias=0.0)
                nc.sync.dma_start(outd.ap()[bg * bgrp:(bg + 1) * bgrp],
                                  y_d[:])

            # ---- h9 . v_h in fp32, natural layout ----
            yh = const.tile([P, NBT], f32, name="yh")
            for bt in range(NBT):
                h9 = work.tile([P, CH], f32, name="h9", tag="h9")
                nc.sync.dma_start(h9[:], h_ap[bt * P:(bt + 1) * P, T - 1, :])
                tmp = work.tile([P, CH], f32, name="tmp9", tag="tmp9")
                nc.vector.tensor_tensor(tmp[:], h9[:], vh_sb[:], ALU.mult)
                nc.vector.tensor_reduce(yh[:, bt:bt + 1], tmp[:],
                                        mybir.AxisListType.X, ALU.add)
            nc.sync.dma_start(outh.ap(), yh[:])

    nc.compile()
    return nc


def _host_prep(inputs):
    W_hh = np.asarray(inputs["W_hh"], np.float32)
    W_ih = np.asarray(inputs["W_ih"], np.float32)
    b = (np.asarray(inputs["b_ih"], np.float32)
         + np.asarray(inputs["b_hh"], np.float32))          # [2048]
    fc1_w = np.asarray(inputs["fc1_w"], np.float32)
    fc2_w = np.asarray(inputs["fc2_w"], np.float32)
    v = (fc2_w @ fc1_w)[0]                                   # [1024]
    c0 = float((np.asarray(inputs["fc1_b"], np.float32) @ fc2_w[0].T)
               .sum() + float(np.asarray(inputs["fc2_b"], np.float32)[0]))

    NK = CH // P
    # w_all[k][r, col] = W_hh[col, 128k + r]
    w_all = np.ascontiguousarray(
        W_hh.T.reshape(NK, P, 4 * H).astype(BF16))
    wih = np.ascontiguousarray(W_ih[:, 0][None, :].astype(BF16))  # [1, 2048]
    bias = np.ascontiguousarray(b.reshape(4 * H // P, P).T.copy())  # [128,16]
    v_d = np.ascontiguousarray(v[:H].reshape(H // P, P).T.copy())   # [128,4]
    v_h = np.ascontiguousarray(np.tile(v[H:][None, :], (P, 1)))     # [128,512]
    return w_all, wih, bias.astype(np.float32), v_d.astype(np.float32), \
        v_h.astype(np.float32), c0


def _install_ntff_shim():
    """Best-effort: recreate antenv.axon_hooks so trace=True can profile."""
    import sys as _sys
    import types as _types
    try:
        import antenv.axon_hooks  # noqa: F401
        return
    except ImportError:
        pass
    try:
        import antenv
        from trn_agent_boot.trn_boot import _ntff_profile_via_ctypes
        hook = _ntff_profile_via_ctypes("/opt/axon/libaxon_pjrt.so")
        mod = _types.ModuleType("antenv.axon_hooks")
        _state = {"hook": hook}
        mod.set_axon_ntff_profile_hook = lambda hk: _state.__setitem__("hook", hk)
        mod.get_axon_ntff_profile_hook = lambda: _state["hook"]
        _sys.modules["antenv.axon_hooks"] = mod
        antenv.axon_hooks = mod
    except Exception:
        pass


def run(inputs, trace=False):
    key = "full"
    if key not in _compiled:
        _compiled[key] = build_nc()
    nc = _compiled[key]

    if trace:
        _install_ntff_shim()

    w_all, wih, bias, v_d, v_h, c0 = _host_prep(inputs)
    h = np.asarray(inputs["h"], np.float32)
    y = np.asarray(inputs["y_seq"], np.float32)

    in_maps = []
    for c in range(N_CORES):
        sl = slice(c * B_LOC, (c + 1) * B_LOC)
        in_maps.append({
            "h": np.ascontiguousarray(h[sl]),
            "yT": np.ascontiguousarray(y[sl].T.astype(BF16)),
            "w_all": w_all, "wih": wih, "bias": bias,
            "v_d": v_d, "v_h": v_h,
        })

    res = run_bass_kernel_spmd(nc, in_maps, core_ids=list(range(N_CORES)),
                               trace=trace)
    outs = []
    for c in range(N_CORES):
        r = res.results[c]
        y_core = (r["out_d"] + r["out_h"].T.reshape(-1) + c0)
        outs.append(y_core.astype(np.float32))
    return np.concatenate(outs)[:, None], res


def kernel(**inputs):
    out, _ = run(inputs, trace=False)
    return out


# revision 7
# speedup vs baseline: 1.0743x; 1.0743x over previous
"""Trainium2 Bass kernel for nn_AttnDecoder (B=8192, T=10, CH=H=512).

Math notes (verified against the jax reference in fp32 to ~3e-6):
  - The attention block is dead code: softmax over a size-1 axis == 1, so
    h1 == ht and attn1/2/3 never affect the output.
  - The LSTM hidden state d never feeds back into the gates (only the cell
    state c does, elementwise), so the only sequential part is
        c_t = sigmoid(f_t) * c_{t-1} + sigmoid(i_t) * tanh(g_t)
    a cheap elementwise recurrence over T=10.
  - o-gate is only needed at t = T-1.
  - fc2(fc1(z)) with no nonlinearity folds into a single vector:
        y = d . v[:H] + h9 . v[H:] + c0,   v = (fc2_w @ fc1_w)^T.

Sharding: batch-parallel over 8 cores (1024 rows each), weights replicated.

Device pipeline per core (all engine writes contiguous — strided SBUF writes
run ~4x slow on cayman):
  1. SWDGE cast-DMA: h fp32 (DRAM) -> bf16 DRAM staging, per (batch-group, t).
  2. HWDGE xbar transpose-DMA: staging -> SBUF hT [ch, batch] bf16 tiles.
  3. PE: per (t, gate): one 4-bank PSUM tile [128, 4*512]; per hid-chunk j a
     K=2 rank-2 matmul folds both the y_t (x) w_ih term and the gate bias
     (rhs rows = [y_t; ones]), then 4 K=128 bf16 matmuls accumulate W @ hT.
  4. ACT: one wide sigmoid/tanh per (t, gate) straight from PSUM (sigmoid and
     tanh live in one table set) -> bf16 SBUF.
  5. DVE: m = si*tg (bf16 2x); c = c*sf + m unrolled over t in fp32.
  6. Final: d = sigma(o)*tanh(c); y_d via fp32 PE dot with v[:H];
     y_h = h9 . v[H:] in fp32 on DVE from naturally-laid-out h9;
     partial outputs summed on the host during unsharding.
"""

import numpy as np
import ml_dtypes

import concourse.bass as bass
import concourse.tile as tile
from concourse import bacc, mybir
from concourse.bass_utils import run_bass_kernel_spmd

BF16 = ml_dtypes.bfloat16

B, T, CH, H = 8192, 10, 512, 512
N_CORES = 8
B_LOC = B // N_CORES            # 1024 batch rows per core
P = 128

_compiled = {}


def build_nc(b_loc=B_LOC, bgrp=512, psum_bufs=2, hT_bufs=2, work_bufs=3):
    NBG = b_loc // bgrp         # batch groups
    NJ = H // P                 # 4 hid chunks
    NK = CH // P                # 4 contraction chunks
    NBT = b_loc // P            # batch tiles for the h9 dot
    GW = NJ * bgrp              # big-tile width (one gate, all hid chunks)
    f32 = mybir.dt.float32
    bf16 = mybir.dt.bfloat16
    AF = mybir.ActivationFunctionType
    ALU = mybir.AluOpType

    nc = bacc.Bacc("TRN2", target_bir_lowering=False, debug=False,
                   num_devices=N_CORES)

    h_in = nc.dram_tensor("h", [b_loc, T, CH], f32, kind="ExternalInput")
    # per t: rhs rows [y_t ; ones] for the K=2 bias/y matmul
    yt_in = nc.dram_tensor("yt_aug", [2, T, b_loc], bf16, kind="ExternalInput")
    # w_all[k][r, col] = W_hh[col, 128k + r] (bf16), col spans i,f,g,o = 2048
    w_in = nc.dram_tensor("w_all", [NK, P, 4 * H], bf16, kind="ExternalInput")
    # rows: [w_ih ; b_ih + b_hh]
    wb_in = nc.dram_tensor("wih_b", [2, 4 * H], bf16, kind="ExternalInput")
    vd_in = nc.dram_tensor("v_d", [P, NJ], f32, kind="ExternalInput")
    vh_in = nc.dram_tensor("v_h", [P, CH], f32, kind="ExternalInput")
    outd = nc.dram_tensor("out_d", [b_loc], f32, kind="ExternalOutput")
    outh = nc.dram_tensor("out_h", [P, NBT], f32, kind="ExternalOutput")

    h_ap = h_in.ap()
    G_I, G_F, G_G, G_O = 0, 1, 2, 3     # gate blocks in the 2048 W columns

    with tile.TileContext(nc) as tc:
        with (
            tc.tile_pool(name="const", bufs=1) as const,
            tc.tile_pool(name="dram", bufs=1, space="DRAM") as dram,
            tc.tile_pool(name="hT", bufs=hT_bufs) as hTp,
            tc.tile_pool(name="work", bufs=work_bufs) as work,
            tc.tile_pool(name="fin", bufs=2) as fin,
            tc.tile_pool(name="psum", bufs=psum_bufs, space="PSUM") as psum,
        ):
            # ---- weights / constants into SBUF ----
            w_sb = []
            for k in range(NK):
                wt = const.tile([P, 4 * H], bf16, name=f"w_sb{k}", tag=f"w{k}")
                nc.sync.dma_start(wt[:], w_in.ap()[k])
                w_sb.append(wt)
            wb_sb = const.tile([2, 4 * H], bf16, name="wb_sb")
            nc.sync.dma_start(wb_sb[:], wb_in.ap())
            vd_sb = const.tile([P, NJ], f32, name="vd_sb")
            nc.sync.dma_start(vd_sb[:], vd_in.ap())
            vh_sb = const.tile([P, CH], f32, name="vh_sb")
            nc.sync.dma_start(vh_sb[:], vh_in.ap())
            yt_sb = const.tile([2, T * b_loc], bf16, name="yt_sb")
            nc.sync.dma_start(
                yt_sb[:], yt_in.ap().rearrange("r t b -> r (t b)"))

            # ---- stage 1: cast h to bf16 in DRAM (SWDGE cast DMA) ----
            staged = [[None] * T for _ in range(NBG)]
            for bg in range(NBG):
                bs = bg * bgrp
                for t in range(T):
                    st = dram.tile([bgrp, CH], bf16, name=f"stg_{bg}_{t}",
                                   tag=f"stg_{bg}_{t}")
                    nc.gpsimd.dma_start(st[:], h_ap[bs:bs + bgrp, t, :])
                    staged[bg][t] = st

            def gate_matmul(gate, hT, t, bg):
                """One 4-bank psum tile covering all NJ chunks of a gate."""
                ps = psum.tile([P, GW], f32, name="ps_big", tag="ps")
                for j in range(NJ):
                    mi = gate * NJ + j
                    nc.tensor.matmul(
                        ps[:, j * bgrp:(j + 1) * bgrp],
                        wb_sb[:, mi * P:(mi + 1) * P],
                        yt_sb[:, t * b_loc + bg * bgrp:
                              t * b_loc + (bg + 1) * bgrp],
                        start=True, stop=False)
                    for k in range(NK):
                        nc.tensor.matmul(
                            ps[:, j * bgrp:(j + 1) * bgrp],
                            w_sb[k][:, mi * P:(mi + 1) * P],
                            hT[k][:],
                            start=False, stop=(k == NK - 1))
                return ps

            # ---- main loop ----
            for bg in range(NBG):
                c_t = const.tile([P, GW], f32, name=f"c_{bg}", tag=f"c{bg}")
                so_t = None
                hT9 = None

                for t in range(T):
                    hT = []
                    for k in range(NK):
                        ht = hTp.tile([P, bgrp], bf16, name=f"hT{k}",
                                      tag=f"hT{k}")
                        nc.sync.dma_start_transpose(
                            ht[:], staged[bg][t][:, k * P:(k + 1) * P])
                        hT.append(ht)
                    if t == T - 1:
                        hT9 = hT

                    ps = gate_matmul(G_I, hT, t, bg)
                    si = work.tile([P, GW], bf16, name="si", tag="si")
                    nc.scalar.activation(si[:], ps[:], AF.Sigmoid)

                    ps = gate_matmul(G_G, hT, t, bg)
                    tg = work.tile([P, GW], bf16, name="tg", tag="tg")
                    nc.scalar.activation(tg[:], ps[:], AF.Tanh)

                    m = work.tile([P, GW], bf16, name="m", tag="m")
                    nc.vector.tensor_tensor(m[:], si[:], tg[:], ALU.mult)

                    if t > 0:
                        ps = gate_matmul(G_F, hT, t, bg)
                        sf = work.tile([P, GW], bf16, name="sf", tag="sf")
                        nc.scalar.activation(sf[:], ps[:], AF.Sigmoid)
                        # c = c * sf + m   (fp32 state)
                        nc.vector.tensor_tensor(c_t[:], c_t[:], sf[:],
                                                ALU.mult)
                        nc.vector.tensor_tensor(c_t[:], c_t[:], m[:],
                                                ALU.add)
                    else:
                        nc.vector.tensor_copy(c_t[:], m[:])

                    if t == T - 1:
                        ps = gate_matmul(G_O, hT, t, bg)
                        so_t = fin.tile([P, GW], f32, name="so", tag="so",
                                        bufs=1)
                        nc.scalar.activation(so_t[:], ps[:], AF.Sigmoid)

                # ---- finalize this batch group ----
                tc9 = fin.tile([P, GW], f32, name="tc9", tag="tc9", bufs=1)
                nc.scalar.activation(tc9[:], c_t[:], AF.Tanh)
                d = fin.tile([P, GW], f32, name="d", tag="d", bufs=1)
                nc.vector.tensor_tensor(d[:], so_t[:], tc9[:], ALU.mult)
                ps_y = psum.tile([1, bgrp], f32, name="ps_y", tag="ps")
                for j in range(NJ):
                    nc.tensor.matmul(ps_y[:], vd_sb[:, j:j + 1],
                                     d[:, j * bgrp:(j + 1) * bgrp],
                                     start=(j == 0), stop=(j == NJ - 1))
                y_d = fin.tile([1, bgrp], f32, name="y_d", tag="y_d")
                nc.scalar.activation(y_d[:], ps_y[:], AF.Copy, bias=0.0)
                nc.sync.dma_start(outd.ap()[bg * bgrp:(bg + 1) * bgrp],
                                  y_d[:])

            # ---- h9 . v_h in fp32, natural layout ----
            yh = const.tile([P, NBT], f32, name="yh")
            for bt in range(NBT):
                h9 = work.tile([P, CH], f32, name="h9", tag="h9")
                nc.sync.dma_start(h9[:], h_ap[bt * P:(bt + 1) * P, T - 1, :])
                tmp = work.tile([P, CH], f32, name="tmp9", tag="tmp9")
                nc.vector.tensor_tensor(tmp[:], h9[:], vh_sb[:], ALU.mult)
                nc.vector.tensor_reduce(yh[:, bt:bt + 1], tmp[:],
                                        mybir.AxisListType.X, ALU.add)
            nc.sync.dma_start(outh.ap(), yh[:])

    nc.compile()
    return nc


def _host_prep(inputs):
    W_hh = np.asarray(inputs["W_hh"], np.float32)
    W_ih = np.asarray(inputs["W_ih"], np.float32)
    b = (np.asarray(inputs["b_ih"], np.float32)
         + np.asarray(inputs["b_hh"], np.float32))          # [2048]
    fc1_w = np.asarray(inputs["fc1_w"], np.float32)
    fc2_w = np.asarray(inputs["fc2_w"], np.float32)
    v = (fc2_w @ fc1_w)[0]                                   # [1024]
    c0 = float(np.asarray(inputs["fc1_b"], np.float32) @ fc2_w[0]
               + np.asarray(inputs["fc2_b"], np.float32)[0])

    NK = CH // P
    w_all = np.ascontiguousarray(W_hh.T.reshape(NK, P, 4 * H).astype(BF16))
    wih_b = np.ascontiguousarray(
        np.stack([W_ih[:, 0], b]).astype(BF16))              # [2, 2048]
    v_d = np.ascontiguousarray(v[:H].reshape(H // P, P).T.copy())   # [128,4]
    v_h = np.ascontiguousarray(np.tile(v[H:][None, :], (P, 1)))     # [128,512]
    return w_all, wih_b, v_d.astype(np.float32), v_h.astype(np.float32), c0


def _install_ntff_shim():
    """Best-effort: recreate antenv.axon_hooks so trace=True can profile."""
    import sys as _sys
    import types as _types
    try:
        import antenv.axon_hooks  # noqa: F401
        return
    except ImportError:
        pass
    try:
        import antenv
        from trn_agent_boot.trn_boot import _ntff_profile_via_ctypes
        hook = _ntff_profile_via_ctypes("/opt/axon/libaxon_pjrt.so")
        mod = _types.ModuleType("antenv.axon_hooks")
        _state = {"hook": hook}
        mod.set_axon_ntff_profile_hook = lambda hk: _state.__setitem__("hook", hk)
        mod.get_axon_ntff_profile_hook = lambda: _state["hook"]
        _sys.modules["antenv.axon_hooks"] = mod
        antenv.axon_hooks = mod
    except Exception:
        pass


def make_in_maps(inputs):
    w_all, wih_b, v_d, v_h, c0 = _host_prep(inputs)
    h = np.asarray(inputs["h"], np.float32)
    y = np.asarray(inputs["y_seq"], np.float32)
    in_maps = []
    for c in range(N_CORES):
        sl = slice(c * B_LOC, (c + 1) * B_LOC)
        yt = np.empty((2, T, B_LOC), BF16)
        yt[0] = y[sl].T.astype(BF16)
        yt[1] = np.ones((T, B_LOC), BF16)
        in_maps.append({
            "h": np.ascontiguousarray(h[sl]),
            "yt_aug": yt,
            "w_all": w_all, "wih_b": wih_b,
            "v_d": v_d, "v_h": v_h,
        })
    return in_maps, c0


def run(inputs, trace=False):
    key = "full"
    if key not in _compiled:
        _compiled[key] = build_nc()
    nc = _compiled[key]

    if trace:
        _install_ntff_shim()

    in_maps, c0 = make_in_maps(inputs)
    res = run_bass_kernel_spmd(nc, in_maps, core_ids=list(range(N_CORES)),
                               trace=trace)
    outs = []
    for c in range(N_CORES):
        r = res.results[c]
        y_core = (r["out_d"] + r["out_h"].T.reshape(-1) + c0)
        outs.append(y_core.astype(np.float32))
    return np.concatenate(outs)[:, None], res


def kernel(**inputs):
    out, _ = run(inputs, trace=False)
    return out


# revision 12
# speedup vs baseline: 1.3773x; 1.2821x over previous
"""Trainium2 Bass kernel for nn_AttnDecoder (B=8192, T=10, CH=H=512).

Math notes (verified against the jax reference in fp32 to ~3e-6):
  - The attention block is dead code: softmax over a size-1 axis == 1, so
    h1 == ht and attn1/2/3 never affect the output.
  - The LSTM hidden state d never feeds back into the gates (only the cell
    state c does, elementwise), so the only sequential part is
        c_t = sigmoid(f_t) * c_{t-1} + sigmoid(i_t) * tanh(g_t)
    a cheap elementwise recurrence over T=10.
  - o-gate is only needed at t = T-1.
  - fc2(fc1(z)) with no nonlinearity folds into a single vector:
        y = d . v[:H] + h9 . v[H:] + c0,   v = (fc2_w @ fc1_w)^T.

Sharding: batch-parallel over 8 cores (1024 rows each), weights replicated.

Device pipeline per core (all engine writes contiguous — strided SBUF writes
run ~4x slow on cayman):
  1. SWDGE cast-DMA: h fp32 (DRAM) -> bf16 DRAM staging, per (batch-group, t).
  2. HWDGE xbar transpose-DMA: staging -> SBUF hT [ch, batch] bf16 tiles.
  3. PE: per (t, gate): one 4-bank PSUM tile [128, 4*512]; per hid-chunk j a
     K=2 rank-2 matmul folds both the y_t (x) w_ih term and the gate bias
     (rhs rows = [y_t; ones]), then 4 K=128 bf16 matmuls accumulate W @ hT.
  4. ACT: one wide sigmoid/tanh per (t, gate) straight from PSUM (sigmoid and
     tanh live in one table set) -> bf16 SBUF.
  5. DVE: m = si*tg (bf16 2x); c = c*sf + m unrolled over t in fp32.
  6. Final: d = sigma(o)*tanh(c); y_d via fp32 PE dot with v[:H];
     y_h = h9 . v[H:] in fp32 on DVE from naturally-laid-out h9;
     partial outputs summed on the host during unsharding.
"""

import numpy as np
import ml_dtypes

import concourse.bass as bass
import concourse.tile as tile
from concourse import bacc, mybir
from concourse.bass_utils import run_bass_kernel_spmd

BF16 = ml_dtypes.bfloat16

B, T, CH, H = 8192, 10, 512, 512
N_CORES = 8
B_LOC = B // N_CORES            # 1024 batch rows per core
P = 128

_compiled = {}


def build_nc(b_loc=B_LOC, bgrp=512, psum_bufs=2, hT_bufs=2, work_bufs=3,
             pack_y=True):
    NBG = b_loc // bgrp         # batch groups
    NJ = H // P                 # 4 hid chunks
    NK = CH // P                # 4 contraction chunks
    NBT = b_loc // P            # batch tiles for the h9 dot
    GW = NJ * bgrp              # big-tile width (one gate, all hid chunks)
    f32 = mybir.dt.float32
    bf16 = mybir.dt.bfloat16
    AF = mybir.ActivationFunctionType
    ALU = mybir.AluOpType

    nc = bacc.Bacc("TRN2", target_bir_lowering=False, debug=False,
                   num_devices=N_CORES)

    h_in = nc.dram_tensor("h", [b_loc, T, CH], f32, kind="ExternalInput")
    # per t: rhs rows [y_t ; ones] for the K=2 bias/y matmul
    yt_in = nc.dram_tensor("yt_aug", [2, T, b_loc], bf16, kind="ExternalInput")
    # w_all[k][r, col] = W_hh[col, 128k + r] (bf16), col spans i,f,g,o = 2048
    w_in = nc.dram_tensor("w_all", [NK, P, 4 * H], bf16, kind="ExternalInput")
    # rows: [w_ih ; b_ih + b_hh]
    wb_in = nc.dram_tensor("wih_b", [2, 4 * H], bf16, kind="ExternalInput")
    vd_in = nc.dram_tensor("v_d", [P, NJ], f32, kind="ExternalInput")
    vh_in = nc.dram_tensor("v_h", [P, CH], f32, kind="ExternalInput")
    outd = nc.dram_tensor("out_d", [b_loc], f32, kind="ExternalOutput")
    outh = nc.dram_tensor("out_h", [P, NBT], f32, kind="ExternalOutput")

    h_ap = h_in.ap()
    G_I, G_F, G_G, G_O = 0, 1, 2, 3     # gate blocks in the 2048 W columns

    with tile.TileContext(nc) as tc:
        with (
            tc.tile_pool(name="const", bufs=1) as const,
            tc.tile_pool(name="dram", bufs=1, space="DRAM") as dram,
            tc.tile_pool(name="hT", bufs=hT_bufs) as hTp,
            tc.tile_pool(name="work", bufs=work_bufs) as work,
            tc.tile_pool(name="fin", bufs=2) as fin,
            tc.tile_pool(name="psum", bufs=psum_bufs, space="PSUM") as psum,
        ):
            # ---- weights / constants into SBUF ----
            w_sb = []
            for k in range(NK):
                wt = const.tile([P, 4 * H], bf16, name=f"w_sb{k}", tag=f"w{k}")
                nc.sync.dma_start(wt[:], w_in.ap()[k])
                w_sb.append(wt)
            vd_sb = const.tile([P, NJ], f32, name="vd_sb")
            nc.sync.dma_start(vd_sb[:], vd_in.ap())
            vh_sb = const.tile([P, CH], f32, name="vh_sb")
            nc.sync.dma_start(vh_sb[:], vh_in.ap())
            # [w_ih ; bias] rows replicated at partitions {32j, 32j+1} for
            # tile_position row-group packing of the K=2 matmuls
            nrep = NJ if pack_y else 1
            wb_sb = const.tile([(nrep - 1) * 32 + 2, 4 * H], bf16,
                               name="wb_sb")
            yt_sb = const.tile([(nrep - 1) * 32 + 2, T * b_loc], bf16,
                               name="yt_sb")
            for r in range(nrep):
                nc.sync.dma_start(wb_sb[32 * r:32 * r + 2, :], wb_in.ap())
                nc.sync.dma_start(
                    yt_sb[32 * r:32 * r + 2, :],
                    yt_in.ap().rearrange("r t b -> r (t b)"))

            # ---- h9 loads (before any transpose DMA: xbar-mode switches
            # serialize against in-flight plain DMAs) ----
            h9_t = []
            for bt in range(NBT):
                h9 = const.tile([P, CH], f32, name=f"h9_{bt}", tag=f"h9_{bt}")
                nc.sync.dma_start(h9[:], h_ap[bt * P:(bt + 1) * P, T - 1, :])
                h9_t.append(h9)

            def gate_matmul(gate, hT, t, bg):
                """One 4-bank psum tile covering all NJ chunks of a gate."""
                ps = psum.tile([P, GW], f32, name="ps_big", tag="ps")
                if pack_y:
                    # NB: correctness requires each j-region to be exactly one
                    # PSUM bank (bgrp == 512 fp32): start=True clears
                    # has_written for the whole bank.
                    for j in range(NJ):
                        mi = gate * NJ + j
                        nc.tensor.matmul(
                            ps[:, j * bgrp:(j + 1) * bgrp],
                            wb_sb[32 * j:32 * j + 2, mi * P:(mi + 1) * P],
                            yt_sb[32 * j:32 * j + 2,
                                  t * b_loc + bg * bgrp:
                                  t * b_loc + (bg + 1) * bgrp],
                            start=True, stop=False,
                            tile_position=(32 * j, 0),
                            skip_group_check=True)
                    for j in range(NJ):
                        mi = gate * NJ + j
                        for k in range(NK):
                            nc.tensor.matmul(
                                ps[:, j * bgrp:(j + 1) * bgrp],
                                w_sb[k][:, mi * P:(mi + 1) * P],
                                hT[k][:],
                                start=False, stop=(k == NK - 1),
                                skip_group_check=True)
                else:
                    ytsl = yt_sb[:, t * b_loc + bg * bgrp:
                                 t * b_loc + (bg + 1) * bgrp]
                    for j in range(NJ):
                        mi = gate * NJ + j
                        nc.tensor.matmul(
                            ps[:, j * bgrp:(j + 1) * bgrp],
                            wb_sb[0:2, mi * P:(mi + 1) * P], ytsl,
                            start=True, stop=False)
                        for k in range(NK):
                            nc.tensor.matmul(
                                ps[:, j * bgrp:(j + 1) * bgrp],
                                w_sb[k][:, mi * P:(mi + 1) * P],
                                hT[k][:],
                                start=False, stop=(k == NK - 1))
                return ps

            # ---- main loop ----
            y_d_t = []
            for bg in range(NBG):
                c_t = const.tile([P, GW], f32, name=f"c_{bg}", tag=f"c{bg}")
                so_t = None
                hT9 = None
                bs = bg * bgrp

                for t in range(T):
                    # stage-cast this (bg, t) then transpose it; interleaved
                    # issue keeps each xbar-mode switch waiting only on the
                    # small preceding chunk
                    st = dram.tile([bgrp, CH], bf16, name=f"stg_{bg}_{t}",
                                   tag=f"stg_{bg}_{t}")
                    nc.gpsimd.dma_start(st[:], h_ap[bs:bs + bgrp, t, :])
                    hT = []
                    for k in range(NK):
                        ht = hTp.tile([P, bgrp], bf16, name=f"hT{k}",
                                      tag=f"hT{k}")
                        nc.sync.dma_start_transpose(
                            ht[:], st[:, k * P:(k + 1) * P])
                        hT.append(ht)
                    if t == T - 1:
                        hT9 = hT

                    ps = gate_matmul(G_I, hT, t, bg)
                    si = work.tile([P, GW], bf16, name="si", tag="si")
                    nc.scalar.activation(si[:], ps[:], AF.Sigmoid)

                    ps = gate_matmul(G_G, hT, t, bg)
                    tg = work.tile([P, GW], bf16, name="tg", tag="tg")
                    nc.scalar.activation(tg[:], ps[:], AF.Tanh)

                    m = work.tile([P, GW], bf16, name="m", tag="m")
                    nc.vector.tensor_tensor(m[:], si[:], tg[:], ALU.mult)

                    if t > 0:
                        ps = gate_matmul(G_F, hT, t, bg)
                        sf = work.tile([P, GW], bf16, name="sf", tag="sf")
                        nc.scalar.activation(sf[:], ps[:], AF.Sigmoid)
                        # c = c * sf + m   (fp32 state)
                        nc.vector.tensor_tensor(c_t[:], c_t[:], sf[:],
                                                ALU.mult)
                        nc.vector.tensor_tensor(c_t[:], c_t[:], m[:],
                                                ALU.add)
                    else:
                        nc.vector.tensor_copy(c_t[:], m[:])

                    if t == T - 1:
                        ps = gate_matmul(G_O, hT, t, bg)
                        so_t = fin.tile([P, GW], f32, name="so", tag="so",
                                        bufs=1)
                        nc.scalar.activation(so_t[:], ps[:], AF.Sigmoid)

                # ---- finalize this batch group ----
                tc9 = fin.tile([P, GW], f32, name="tc9", tag="tc9", bufs=1)
                nc.scalar.activation(tc9[:], c_t[:], AF.Tanh)
                d = fin.tile([P, GW], f32, name="d", tag="d", bufs=1)
                nc.vector.tensor_tensor(d[:], so_t[:], tc9[:], ALU.mult)
                ps_y = psum.tile([1, bgrp], f32, name="ps_y", tag="ps")
                for j in range(NJ):
                    nc.tensor.matmul(ps_y[:], vd_sb[:, j:j + 1],
                                     d[:, j * bgrp:(j + 1) * bgrp],
                                     start=(j == 0), stop=(j == NJ - 1))
                y_d = fin.tile([1, bgrp], f32, name="y_d", tag=f"y_d{bg}",
                               bufs=1)
                nc.scalar.activation(y_d[:], ps_y[:], AF.Copy, bias=0.0)
                y_d_t.append(y_d)

            # ---- h9 . v_h in fp32, natural layout (DVE has slack) ----
            yh = const.tile([P, NBT], f32, name="yh")
            for bt in range(NBT):
                tmp = work.tile([P, CH], f32, name="tmp9", tag="tmp9")
                nc.vector.tensor_tensor(tmp[:], h9_t[bt][:], vh_sb[:],
                                        ALU.mult)
                nc.vector.tensor_reduce(yh[:, bt:bt + 1], tmp[:],
                                        mybir.AxisListType.X, ALU.add)

            # ---- outputs last (plain DMAs after all transpose DMAs) ----
            for bg in range(NBG):
                nc.sync.dma_start(outd.ap()[bg * bgrp:(bg + 1) * bgrp],
                                  y_d_t[bg][:])
            nc.sync.dma_start(outh.ap(), yh[:])

    nc.compile()
    return nc


def _host_prep(inputs):
    W_hh = np.asarray(inputs["W_hh"], np.float32)
    W_ih = np.asarray(inputs["W_ih"], np.float32)
    b = (np.asarray(inputs["b_ih"], np.float32)
         + np.asarray(inputs["b_hh"], np.float32))          # [2048]
    fc1_w = np.asarray(inputs["fc1_w"], np.float32)
    fc2_w = np.asarray(inputs["fc2_w"], np.float32)
    v = (fc2_w @ fc1_w)[0]                                   # [1024]
    c0 = float(np.asarray(inputs["fc1_b"], np.float32) @ fc2_w[0]
               + np.asarray(inputs["fc2_b"], np.float32)[0])

    NK = CH // P
    w_all = np.ascontiguousarray(W_hh.T.reshape(NK, P, 4 * H).astype(BF16))
    wih_b = np.ascontiguousarray(
        np.stack([W_ih[:, 0], b]).astype(BF16))              # [2, 2048]
    v_d = np.ascontiguousarray(v[:H].reshape(H // P, P).T.copy())   # [128,4]
    v_h = np.ascontiguousarray(np.tile(v[H:][None, :], (P, 1)))     # [128,512]
    return w_all, wih_b, v_d.astype(np.float32), v_h.astype(np.float32), c0


def _install_ntff_shim():
    """Best-effort: recreate antenv.axon_hooks so trace=True can profile."""
    import sys as _sys
    import types as _types
    try:
        import antenv.axon_hooks  # noqa: F401
        return
    except ImportError:
        pass
    try:
        import antenv
        from trn_agent_boot.trn_boot import _ntff_profile_via_ctypes
        hook = _ntff_profile_via_ctypes("/opt/axon/libaxon_pjrt.so")
        mod = _types.ModuleType("antenv.axon_hooks")
        _state = {"hook": hook}
        mod.set_axon_ntff_profile_hook = lambda hk: _state.__setitem__("hook", hk)
        mod.get_axon_ntff_profile_hook = lambda: _state["hook"]
        _sys.modules["antenv.axon_hooks"] = mod
        antenv.axon_hooks = mod
    except Exception:
        pass


def make_in_maps(inputs):
    w_all, wih_b, v_d, v_h, c0 = _host_prep(inputs)
    h = np.asarray(inputs["h"], np.float32)
    y = np.asarray(inputs["y_seq"], np.float32)
    in_maps = []
    for c in range(N_CORES):
        sl = slice(c * B_LOC, (c + 1) * B_LOC)
        yt = np.empty((2, T, B_LOC), BF16)
        yt[0] = y[sl].T.astype(BF16)
        yt[1] = np.ones((T, B_LOC), BF16)
        in_maps.append({
            "h": np.ascontiguousarray(h[sl]),
            "yt_aug": yt,
            "w_all": w_all, "wih_b": wih_b,
            "v_d": v_d, "v_h": v_h,
        })
    return in_maps, c0


def run(inputs, trace=False):
    key = "full"
    if key not in _compiled:
        _compiled[key] = build_nc()
    nc = _compiled[key]

    if trace:
        _install_ntff_shim()

    in_maps, c0 = make_in_maps(inputs)
    res = run_bass_kernel_spmd(nc, in_maps, core_ids=list(range(N_CORES)),
                               trace=trace)
    outs = []
    for c in range(N_CORES):
        r = res.results[c]
        y_core = (r["out_d"] + r["out_h"].T.reshape(-1) + c0)
        outs.append(y_core.astype(np.float32))
    return np.concatenate(outs)[:, None], res


def kernel(**inputs):
    out, _ = run(inputs, trace=False)
    return out


# revision 14
# speedup vs baseline: 1.4616x; 1.0612x over previous
"""Trainium2 Bass kernel for nn_AttnDecoder (B=8192, T=10, CH=H=512).

Math notes (verified against the jax reference in fp32 to ~3e-6):
  - The attention block is dead code: softmax over a size-1 axis == 1, so
    h1 == ht and attn1/2/3 never affect the output.
  - The LSTM hidden state d never feeds back into the gates (only the cell
    state c does, elementwise), so the only sequential part is
        c_t = sigmoid(f_t) * c_{t-1} + sigmoid(i_t) * tanh(g_t)
    a cheap elementwise recurrence over T=10.
  - o-gate is only needed at t = T-1.
  - fc2(fc1(z)) with no nonlinearity folds into a single vector:
        y = d . v[:H] + h9 . v[H:] + c0,   v = (fc2_w @ fc1_w)^T.

Sharding: batch-parallel over 8 cores (1024 rows each), weights replicated.

Device pipeline per core (all engine writes contiguous — strided SBUF writes
run ~4x slow on cayman):
  1. SWDGE cast-DMA: h fp32 (DRAM) -> bf16 DRAM staging, per (batch-group, t).
  2. HWDGE xbar transpose-DMA: staging -> SBUF hT [ch, batch] bf16 tiles.
  3. PE: per (t, gate): one 4-bank PSUM tile [128, 4*512]; per hid-chunk j a
     K=2 rank-2 matmul folds both the y_t (x) w_ih term and the gate bias
     (rhs rows = [y_t; ones]), then 4 K=128 bf16 matmuls accumulate W @ hT.
  4. ACT: one wide sigmoid/tanh per (t, gate) straight from PSUM (sigmoid and
     tanh live in one table set) -> bf16 SBUF.
  5. DVE: m = si*tg (bf16 2x); c = c*sf + m unrolled over t in fp32.
  6. Final: d = sigma(o)*tanh(c); y_d via fp32 PE dot with v[:H];
     y_h = h9 . v[H:] in fp32 on DVE from naturally-laid-out h9;
     partial outputs summed on the host during unsharding.
"""

import numpy as np
import ml_dtypes

import concourse.bass as bass
import concourse.tile as tile
from concourse import bacc, mybir
from concourse.bass_utils import run_bass_kernel_spmd

BF16 = ml_dtypes.bfloat16

B, T, CH, H = 8192, 10, 512, 512
N_CORES = 8
B_LOC = B // N_CORES            # 1024 batch rows per core
P = 128

_compiled = {}


def build_nc(b_loc=B_LOC, bgrp=512, psum_bufs=2, hT_bufs=4, work_bufs=3,
             pack_y=True):
    NBG = b_loc // bgrp         # batch groups
    NJ = H // P                 # 4 hid chunks
    NK = CH // P                # 4 contraction chunks
    NBT = b_loc // P            # batch tiles for the h9 dot
    GW = NJ * bgrp              # big-tile width (one gate, all hid chunks)
    f32 = mybir.dt.float32
    bf16 = mybir.dt.bfloat16
    AF = mybir.ActivationFunctionType
    ALU = mybir.AluOpType

    nc = bacc.Bacc("TRN2", target_bir_lowering=False, debug=False,
                   num_devices=N_CORES)

    h_in = nc.dram_tensor("h", [b_loc, T, CH], f32, kind="ExternalInput")
    # per t: rhs rows [y_t ; ones] for the K=2 bias/y matmul
    yt_in = nc.dram_tensor("yt_aug", [2, T, b_loc], bf16, kind="ExternalInput")
    # w_all[k][r, col] = W_hh[col, 128k + r] (bf16), col spans i,f,g,o = 2048
    w_in = nc.dram_tensor("w_all", [NK, P, 4 * H], bf16, kind="ExternalInput")
    # rows: [w_ih ; b_ih + b_hh]
    wb_in = nc.dram_tensor("wih_b", [2, 4 * H], bf16, kind="ExternalInput")
    vd_in = nc.dram_tensor("v_d", [P, NJ], f32, kind="ExternalInput")
    vh_in = nc.dram_tensor("v_h", [P, CH], f32, kind="ExternalInput")
    outd = nc.dram_tensor("out_d", [b_loc], f32, kind="ExternalOutput")
    outh = nc.dram_tensor("out_h", [P, NBT], f32, kind="ExternalOutput")

    h_ap = h_in.ap()
    G_I, G_F, G_G, G_O = 0, 1, 2, 3     # gate blocks in the 2048 W columns

    with tile.TileContext(nc) as tc:
        with (
            tc.tile_pool(name="const", bufs=1) as const,
            tc.tile_pool(name="dram", bufs=1, space="DRAM") as dram,
            tc.tile_pool(name="hT", bufs=hT_bufs) as hTp,
            tc.tile_pool(name="work", bufs=work_bufs) as work,
            tc.tile_pool(name="fin", bufs=2) as fin,
            tc.tile_pool(name="psum", bufs=psum_bufs, space="PSUM") as psum,
        ):
            # ---- weights / constants into SBUF ----
            w_sb = []
            for k in range(NK):
                wt = const.tile([P, 4 * H], bf16, name=f"w_sb{k}", tag=f"w{k}")
                nc.sync.dma_start(wt[:], w_in.ap()[k])
                w_sb.append(wt)
            vd_sb = const.tile([P, NJ], f32, name="vd_sb")
            nc.sync.dma_start(vd_sb[:], vd_in.ap())
            vh_sb = const.tile([P, CH], f32, name="vh_sb")
            nc.sync.dma_start(vh_sb[:], vh_in.ap())
            # [w_ih ; bias] rows replicated at partitions {32j, 32j+1} for
            # tile_position row-group packing of the K=2 matmuls
            nrep = NJ if pack_y else 1
            wb_sb = const.tile([(nrep - 1) * 32 + 2, 4 * H], bf16,
                               name="wb_sb")
            yt_sb = const.tile([(nrep - 1) * 32 + 2, T * b_loc], bf16,
                               name="yt_sb")
            for r in range(nrep):
                nc.sync.dma_start(wb_sb[32 * r:32 * r + 2, :], wb_in.ap())
                nc.sync.dma_start(
                    yt_sb[32 * r:32 * r + 2, :],
                    yt_in.ap().rearrange("r t b -> r (t b)"))

            h9_t = [None] * NBT
            yh = const.tile([P, NBT], f32, name="yh")

            def gate_matmul(gate, hT, t, bg):
                """One 4-bank psum tile covering all NJ chunks of a gate."""
                ps = psum.tile([P, GW], f32, name="ps_big", tag="ps")
                if pack_y:
                    # NB: correctness requires each j-region to be exactly one
                    # PSUM bank (bgrp == 512 fp32): start=True clears
                    # has_written for the whole bank.
                    for j in range(NJ):
                        mi = gate * NJ + j
                        nc.tensor.matmul(
                            ps[:, j * bgrp:(j + 1) * bgrp],
                            wb_sb[32 * j:32 * j + 2, mi * P:(mi + 1) * P],
                            yt_sb[32 * j:32 * j + 2,
                                  t * b_loc + bg * bgrp:
                                  t * b_loc + (bg + 1) * bgrp],
                            start=True, stop=False,
                            tile_position=(32 * j, 0),
                            skip_group_check=True)
                    for j in range(NJ):
                        mi = gate * NJ + j
                        for k in range(NK):
                            nc.tensor.matmul(
                                ps[:, j * bgrp:(j + 1) * bgrp],
                                w_sb[k][:, mi * P:(mi + 1) * P],
                                hT[k][:],
                                start=False, stop=(k == NK - 1),
                                skip_group_check=True)
                else:
                    ytsl = yt_sb[:, t * b_loc + bg * bgrp:
                                 t * b_loc + (bg + 1) * bgrp]
                    for j in range(NJ):
                        mi = gate * NJ + j
                        nc.tensor.matmul(
                            ps[:, j * bgrp:(j + 1) * bgrp],
                            wb_sb[0:2, mi * P:(mi + 1) * P], ytsl,
                            start=True, stop=False)
                        for k in range(NK):
                            nc.tensor.matmul(
                                ps[:, j * bgrp:(j + 1) * bgrp],
                                w_sb[k][:, mi * P:(mi + 1) * P],
                                hT[k][:],
                                start=False, stop=(k == NK - 1))
                return ps

            # ---- main loop ----
            y_d_t = []
            c_bg = []
            so_bg = []
            for bg in range(NBG):
                c_t = const.tile([P, GW], f32, name=f"c_{bg}", tag=f"c{bg}")
                so_t = None
                bs = bg * bgrp

                for t in range(T):
                    # stage-cast this (bg, t) then transpose it; interleaved
                    # issue keeps each xbar-mode switch waiting only on the
                    # small preceding chunk
                    st = dram.tile([bgrp, CH], bf16, name=f"stg_{bg}_{t}",
                                   tag=f"stg_{bg}_{t}")
                    nc.gpsimd.dma_start(st[:], h_ap[bs:bs + bgrp, t, :])
                    if t == T - 1:
                        # h9 fp32 loads ride the same xbar passthrough window
                        # as this cast (no extra mode transition)
                        for q in range(bgrp // P):
                            bt = (bs + q * P) // P
                            h9 = const.tile([P, CH], f32, name=f"h9_{bt}",
                                            tag=f"h9_{bt}")
                            nc.sync.dma_start(
                                h9[:], h_ap[bt * P:(bt + 1) * P, T - 1, :])
                            h9_t[bt] = h9
                    hT = []
                    for k in range(NK):
                        ht = hTp.tile([P, bgrp], bf16, name=f"hT{k}",
                                      tag=f"hT{k}")
                        nc.sync.dma_start_transpose(
                            ht[:], st[:, k * P:(k + 1) * P])
                        hT.append(ht)

                    ps = gate_matmul(G_I, hT, t, bg)
                    si = work.tile([P, GW], bf16, name="si", tag="si")
                    nc.scalar.activation(si[:], ps[:], AF.Sigmoid)

                    ps = gate_matmul(G_G, hT, t, bg)
                    tg = work.tile([P, GW], bf16, name="tg", tag="tg")
                    nc.scalar.activation(tg[:], ps[:], AF.Tanh)

                    m = work.tile([P, GW], bf16, name="m", tag="m")
                    nc.vector.tensor_tensor(m[:], si[:], tg[:], ALU.mult)

                    if t > 0:
                        ps = gate_matmul(G_F, hT, t, bg)
                        sf = work.tile([P, GW], bf16, name="sf", tag="sf")
                        nc.scalar.activation(sf[:], ps[:], AF.Sigmoid)
                        # c = c * sf + m   (fp32 state)
                        nc.vector.tensor_tensor(c_t[:], c_t[:], sf[:],
                                                ALU.mult)
                        nc.vector.tensor_tensor(c_t[:], c_t[:], m[:],
                                                ALU.add)
                    else:
                        nc.vector.tensor_copy(c_t[:], m[:])

                    if t == T - 1:
                        ps = gate_matmul(G_O, hT, t, bg)
                        so_t = fin.tile([P, GW], f32, name="so",
                                        tag=f"so{bg}", bufs=1)
                        nc.scalar.activation(so_t[:], ps[:], AF.Sigmoid)

                c_bg.append(c_t)
                so_bg.append(so_t)
                # h9 . v_h for this group's rows (fp32, natural layout; DVE
                # has slack while the other group computes)
                for q in range(bgrp // P):
                    bt = (bs + q * P) // P
                    tmp = work.tile([P, CH], f32, name="tmp9", tag="tmp9")
                    nc.vector.tensor_tensor(tmp[:], h9_t[bt][:], vh_sb[:],
                                            ALU.mult)
                    nc.vector.tensor_reduce(yh[:, bt:bt + 1], tmp[:],
                                            mybir.AxisListType.X, ALU.add)

            # ---- finalize both batch groups (deferred past the main loops
            # so the PSUM pool never stalls the next group's matmuls) ----
            for bg in range(NBG):
                tc9 = fin.tile([P, GW], f32, name="tc9", tag="tc9", bufs=2)
                nc.scalar.activation(tc9[:], c_bg[bg][:], AF.Tanh)
                d = fin.tile([P, GW], f32, name="d", tag="d", bufs=2)
                nc.vector.tensor_tensor(d[:], so_bg[bg][:], tc9[:], ALU.mult)
                ps_y = psum.tile([1, bgrp], f32, name="ps_y", tag="ps")
                for j in range(NJ):
                    nc.tensor.matmul(ps_y[:], vd_sb[:, j:j + 1],
                                     d[:, j * bgrp:(j + 1) * bgrp],
                                     start=(j == 0), stop=(j == NJ - 1))
                y_d = fin.tile([1, bgrp], f32, name="y_d", tag=f"y_d{bg}",
                               bufs=1)
                nc.scalar.activation(y_d[:], ps_y[:], AF.Copy, bias=0.0)
                y_d_t.append(y_d)

            # ---- outputs last (plain DMAs after all transpose DMAs) ----
            for bg in range(NBG):
                nc.sync.dma_start(outd.ap()[bg * bgrp:(bg + 1) * bgrp],
                                  y_d_t[bg][:])
            nc.sync.dma_start(outh.ap(), yh[:])

    nc.compile()
    return nc


def _host_prep(inputs):
    W_hh = np.asarray(inputs["W_hh"], np.float32)
    W_ih = np.asarray(inputs["W_ih"], np.float32)
    b = (np.asarray(inputs["b_ih"], np.float32)
         + np.asarray(inputs["b_hh"], np.float32))          # [2048]
    fc1_w = np.asarray(inputs["fc1_w"], np.float32)
    fc2_w = np.asarray(inputs["fc2_w"], np.float32)
    v = (fc2_w @ fc1_w)[0]                                   # [1024]
    c0 = float(np.asarray(inputs["fc1_b"], np.float32) @ fc2_w[0]
               + np.asarray(inputs["fc2_b"], np.float32)[0])

    NK = CH // P
    w_all = np.ascontiguousarray(W_hh.T.reshape(NK, P, 4 * H).astype(BF16))
    wih_b = np.ascontiguousarray(
        np.stack([W_ih[:, 0], b]).astype(BF16))              # [2, 2048]
    v_d = np.ascontiguousarray(v[:H].reshape(H // P, P).T.copy())   # [128,4]
    v_h = np.ascontiguousarray(np.tile(v[H:][None, :], (P, 1)))     # [128,512]
    return w_all, wih_b, v_d.astype(np.float32), v_h.astype(np.float32), c0


def _install_ntff_shim():
    """Best-effort: recreate antenv.axon_hooks so trace=True can profile."""
    import sys as _sys
    import types as _types
    try:
        import antenv.axon_hooks  # noqa: F401
        return
    except ImportError:
        pass
    try:
        import antenv
        from trn_agent_boot.trn_boot import _ntff_profile_via_ctypes
        hook = _ntff_profile_via_ctypes("/opt/axon/libaxon_pjrt.so")
        mod = _types.ModuleType("antenv.axon_hooks")
        _state = {"hook": hook}
        mod.set_axon_ntff_profile_hook = lambda hk: _state.__setitem__("hook", hk)
        mod.get_axon_ntff_profile_hook = lambda: _state["hook"]
        _sys.modules["antenv.axon_hooks"] = mod
        antenv.axon_hooks = mod
    except Exception:
        pass


def make_in_maps(inputs):
    w_all, wih_b, v_d, v_h, c0 = _host_prep(inputs)
    h = np.asarray(inputs["h"], np.float32)
    y = np.asarray(inputs["y_seq"], np.float32)
    in_maps = []
    for c in range(N_CORES):
        sl = slice(c * B_LOC, (c + 1) * B_LOC)
        yt = np.empty((2, T, B_LOC), BF16)
        yt[0] = y[sl].T.astype(BF16)
        yt[1] = np.ones((T, B_LOC), BF16)
        in_maps.append({
            "h": np.ascontiguousarray(h[sl]),
            "yt_aug": yt,
            "w_all": w_all, "wih_b": wih_b,
            "v_d": v_d, "v_h": v_h,
        })
    return in_maps, c0


def run(inputs, trace=False):
    key = "full"
    if key not in _compiled:
        _compiled[key] = build_nc()
    nc = _compiled[key]

    if trace:
        _install_ntff_shim()

    in_maps, c0 = make_in_maps(inputs)
    res = run_bass_kernel_spmd(nc, in_maps, core_ids=list(range(N_CORES)),
                               trace=trace)
    outs = []
    for c in range(N_CORES):
        r = res.results[c]
        y_core = (r["out_d"] + r["out_h"].T.reshape(-1) + c0)
        outs.append(y_core.astype(np.float32))
    return np.concatenate(outs)[:, None], res


def kernel(**inputs):
    out, _ = run(inputs, trace=False)
    return out
